# revision 26
# baseline (speedup 1.0000x reference)
"""Trainium2 Bass kernel for nn_DataEmbedding_Stats.

Computation: rolling-window stats (window=24, replicate-padded) over
x (B,S,7) -> 35 features -> circular conv1d(k=3) -> (B,S,512).

Strategy (8 NeuronCores, data parallel over batch, 4 batches/core),
slab-pipelined so stats/relayout overlap the matmul+output phase:
 - 4 slabs of 1024 seq; per slab X [128, 279] f32 with partition
   32j + 7b + c (j = 256-seq chunk, b = local batch, c = channel),
   built from contiguous staged loads + PE transposes.
 - rolling sum/sumsq/max/min via log-doubling shifted ops, split
   DVE (sum/sq/var) + Pool (max/min); finals write bf16 into one
   STK [128, 5*279] tile (stat-major blocks).
 - mini-stats prologue computes stats at seq 4094/4095 (the circular
   wrap cols) from a 25-seq load so slab 0 has no dependency on slab 3.
 - relayout: one 3-dim-AP DMA per (stat, slab) STK -> ST2A [28, 5*4104]
   bf16 (col = 4104*t + m, m-2 = seq mod 4096); F3 [106, 2050] per
   (batch, 2048-seq pair) via one 3-dim-AP DMA per (b, tap).
 - conv as matmul: per 128 positions out[128,512] = F3 slice.T @ Wt
   (bf16), bias as ones-row contraction; PSUM -> f16 stage copies split
   DVE/ACT/Pool; f16 output DMA (host upcasts to f32).
"""

import numpy as np

try:
    import concourse.bass as bass  # noqa: F401
except ImportError:
    import sys

    for _p in ("/opt/trn_rl_repo", "/root/.axon_site/_ro/trn_rl_repo"):
        if _p not in sys.path:
            sys.path.insert(0, _p)

B, S, C, W, D = 32, 4096, 7, 24, 512
NCORES = 8
BSH = B // NCORES          # batches per core
NF = 5 * C                 # 35 features
K = 3 * NF + 1             # 106 contraction rows (ones row last)
HALO = W - 1               # 23
NSLAB = 4
SLAB = S // NSLAB          # 1024
NJ = 4                     # 256-seq chunks per slab
CH = SLAB // NJ            # 256
XW = CH + HALO             # 279
STW = 4104                 # per-stat block width in ST2A
F3W = 2310                 # F3 cols (shared width for both mm groups)
NT_P = (14, 18)            # output tiles per (batch, group)
SOFF = (0, 1792)           # output seq offset per group
MW = 25                    # mini-stats window load (seq 4071..4095)

_CACHE = {}


def _build():
    import concourse.bacc as bacc
    import concourse.tile as tile
    from concourse import mybir

    f32 = mybir.dt.float32
    bf16 = mybir.dt.bfloat16
    f16 = mybir.dt.float16
    Alu = mybir.AluOpType
    Act = mybir.ActivationFunctionType

    nc = bacc.Bacc(
        "TRN2",
        target_bir_lowering=False,
        debug=False,
        enable_asserts=False,
        num_devices=NCORES,
    )

    x_d = nc.dram_tensor("x", (BSH, C, S), f32, kind="ExternalInput")
    wt_d = nc.dram_tensor("wt", (K, D), bf16, kind="ExternalInput")
    ones_d = nc.dram_tensor("ones", (1, F3W), bf16, kind="ExternalInput")
    y_d = nc.dram_tensor("y", (BSH, S, D), f16, kind="ExternalOutput")

    with tile.TileContext(nc) as tc:
        with (
            tc.tile_pool(name="const", bufs=1) as pco,
            tc.tile_pool(name="xp", bufs=2) as pxx,
            tc.tile_pool(name="scr", bufs=2) as pscr,
            tc.tile_pool(name="stk", bufs=2) as pstk,
            tc.tile_pool(name="st2", bufs=1) as pst2,
            tc.tile_pool(name="f3p", bufs=8) as pf3,
            tc.tile_pool(name="psum", bufs=4, space="PSUM") as pps,
            tc.tile_pool(name="outp", bufs=4) as pout,
        ):
            wt = pco.tile([K, D], bf16, tag="wt")
            nc.sync.dma_start(wt[:], wt_d.ap())

            ST2A = pst2.tile([32, 5 * STW], bf16, tag="ST2A")

            def chains(Xt, E, stk, swid, base, eng_mm):
                """Rolling stats on Xt [P, E] (valid outputs at cols>=23);
                finals -> bf16 stk, stat t block at col swid*t + base + col.
                eng_mm: engine for the max/min chains."""
                P = Xt.shape[0]
                T1 = pscr.tile([128, E], f32, tag=f"T1_{E}")
                T2 = pscr.tile([128, E], f32, tag=f"T2_{E}")
                T3 = pscr.tile([128, E], f32, tag=f"T3_{E}")
                T4 = pscr.tile([128, E], f32, tag=f"T4_{E}")
                S24 = pscr.tile([128, E], f32, tag=f"S24_{E}")

                def tt(eng, dst, d0, a, a0, bs, b0, op):
                    eng.tensor_tensor(
                        dst[0:P, d0:E], a[0:P, a0 : a0 + E - d0],
                        bs[0:P, b0 : b0 + E - d0], op,
                    )

                def blk(t):
                    return stk[0:P, swid * t + base + HALO : swid * t + base + E]

                v = nc.vector
                g = eng_mm
                # raw x (bf16 cast)
                nc.scalar.copy(blk(0), Xt[0:P, HALO:E])
                # sum chain -> S24 f32, then bf16 copy (mean; /24 in weights)
                tt(v, T1, 1, Xt, 1, Xt, 0, Alu.add)
                tt(v, T2, 3, T1, 3, T1, 1, Alu.add)
                tt(v, T3, 7, T2, 7, T2, 3, Alu.add)
                tt(v, T1, 15, T3, 15, T3, 7, Alu.add)
                tt(v, S24, 23, T1, 23, T3, 7, Alu.add)
                nc.scalar.copy(blk(1), S24[0:P, HALO:E])
                # squares chain -> T2 = SQ24
                nc.scalar.square(T4[0:P, 0:E], Xt[0:P, 0:E])
                tt(v, T1, 1, T4, 1, T4, 0, Alu.add)
                tt(v, T2, 3, T1, 3, T1, 1, Alu.add)
                tt(v, T3, 7, T2, 7, T2, 3, Alu.add)
                tt(v, T1, 15, T3, 15, T3, 7, Alu.add)
                tt(v, T2, 23, T1, 23, T3, 7, Alu.add)
                # std = sqrt(max(SQ24 - S24^2/24, 0)/23) -> bf16
                nc.scalar.activation(
                    T4[0:P, HALO:E], S24[0:P, HALO:E], Act.Square, 0.0,
                    float(W**-0.5),
                )
                tt(v, T3, 23, T2, 23, T4, 23, Alu.subtract)
                nc.vector.tensor_scalar(
                    T2[0:P, HALO:E], T3[0:P, HALO:E], 0.0, None, Alu.max
                )
                nc.scalar.activation(
                    blk(4), T2[0:P, HALO:E], Act.Sqrt, 0.0, 1.0 / (W - 1)
                )
                # max chain -> bf16
                tt(g, T1, 1, Xt, 1, Xt, 0, Alu.max)
                tt(g, T3, 3, T1, 3, T1, 1, Alu.max)
                tt(g, T1, 7, T3, 7, T3, 3, Alu.max)
                tt(g, T3, 15, T1, 15, T1, 7, Alu.max)
                g.tensor_tensor(
                    blk(2), T3[0:P, HALO:E], T1[0:P, 7 : E - 16], Alu.max
                )
                # min chain -> bf16
                tt(g, T1, 1, Xt, 1, Xt, 0, Alu.min)
                tt(g, T3, 3, T1, 3, T1, 1, Alu.min)
                tt(g, T1, 7, T3, 7, T3, 3, Alu.min)
                tt(g, T3, 15, T1, 15, T1, 7, Alu.min)
                g.tensor_tensor(
                    blk(3), T3[0:P, HALO:E], T1[0:P, 7 : E - 16], Alu.min
                )

            # ---------------- mini-stats for circular wrap (seq 4094/4095)
            MXW = pscr.tile([28, MW], f32, tag="MXW")
            nc.scalar.dma_start(
                MXW[:],
                x_d.ap()[:, :, S - MW : S].rearrange("b c q -> (b c) q"),
            )
            MSTK = pscr.tile([28, 10], bf16, tag="MSTK")  # stat t at cols 2t
            chains(MXW, MW, MSTK, 2, -HALO, nc.vector)

            # ---------------- per-slab processing
            X = [None] * NSLAB
            STK = [None] * NSLAB

            def load_slab(n):
                # x is host-transposed to (BSH, C, S): each chunk row is a
                # contiguous 279-f32 run (halo included in the read range)
                Xn = pxx.tile([128, XW], f32, tag="X", name=f"X_{n}")
                X[n] = Xn
                for j in range(NJ):
                    s0 = SLAB * n + CH * j
                    eng = (nc.gpsimd, nc.sync, nc.gpsimd, nc.scalar)[j % 4]
                    if n == 0 and j == 0:
                        eng.dma_start(
                            Xn[0:28, HALO:XW],
                            x_d.ap()[:, :, 0:CH].rearrange("b c q -> (b c) q"),
                        )
                        # replicate x[b,0,c] into halo cols 0..22
                        nc.vector.tensor_scalar(
                            Xn[0:28, 0:HALO],
                            Xn[0:28, HALO : 2 * HALO],
                            0.0,
                            Xn[0:28, HALO : HALO + 1],
                            Alu.mult,
                            Alu.add,
                        )
                    else:
                        eng.dma_start(
                            Xn[32 * j : 32 * j + 28, :],
                            x_d.ap()[
                                :, :, s0 - HALO : s0 + CH
                            ].rearrange("b c q -> (b c) q"),
                        )

            def stats_slab(n):
                stk = pstk.tile([128, 5 * XW], bf16, tag="STK", name=f"STK_{n}")
                STK[n] = stk
                chains(X[n], XW, stk, XW, 0, nc.vector)

            def relay_slab(n):
                stk = STK[n]
                # one DMA per stat: STK[32j+g, XW*t+23+q] ->
                #   ST2A[g, STW*t + 2 + 1024n + 256j + q]
                for j in range(NJ):
                    # one DMA per chunk covering all 5 stats: plain
                    # partition slice (dep-tracking safe), t-stride in cols
                    src = stk[32 * j : 32 * j + 28, :].rearrange(
                        "g (t m) -> g t m", m=XW
                    )[:, :, HALO:XW]
                    c0 = 2 + SLAB * n + CH * j
                    dst = ST2A[0:28, :].rearrange("g (t m) -> g t m", m=STW)[
                        :, :, c0 : c0 + CH
                    ]
                    eng = (nc.sync, nc.scalar)[(n + j) % 2]
                    eng.dma_start(dst, src)
                if n == 0:
                    # high wrap: seq 0,1 -> cols STW*t + 4098..4099
                    nc.sync.dma_start(
                        ST2A[0:28, :].rearrange("g (t m) -> g t m", m=STW)[
                            :, :, S + 2 : S + 4
                        ],
                        stk[0:28, :].rearrange("g (t m) -> g t m", m=XW)[
                            :, :, HALO : HALO + 2
                        ],
                    )

            def wrap_low():
                # mini-stats: seq 4094/4095 -> cols STW*t + 0..1
                nc.scalar.dma_start(
                    ST2A[0:28, :].rearrange("g (t m) -> g t m", m=STW)[:, :, 0:2],
                    MSTK[:, :].rearrange("g (t m) -> g t m", m=2),
                )

            def build_f3(b, P):
                f3 = pf3.tile([K, F3W], bf16, tag="F3", name=f"f3_{b}_{P}")
                nc.gpsimd.dma_start(f3[K - 1 : K, :], ones_d.ap())
                for k in range(3):
                    # F3 row 35k + 5c + t (channel-major; wt permuted to match)
                    # col q <- ST2A col SOFF[P] + q + k; group 0 only uses
                    # q<=1921 (reads past that hit pad cols, never consumed)
                    src = ST2A[7 * b : 7 * b + 7, :].rearrange(
                        "c (t m) -> c t m", m=STW
                    )[:, :, SOFF[P] + k : SOFF[P] + k + F3W - 2]
                    dst = f3[35 * k : 35 * k + 35, 0 : F3W - 2]
                    eng = (nc.sync, nc.gpsimd, nc.scalar)[k]
                    eng.dma_start(dst, src)
                return f3

            def mm_pair(P, f3s):
                # per 256-seq pair block pi: even-seq and odd-seq matmuls
                # (stride-2 lhsT cols) -> psum halves -> one copy; stage
                # partition row p holds y[.., s0+2p] ++ y[.., s0+2p+1]
                # so output descriptors are 2 KB
                nt = NT_P[P]
                for b in range(BSH):
                    f3 = f3s[b]
                    stage = pout.tile([128, nt * D], f16, tag="stage")
                    for pi in range(nt // 2):
                        ps = pps.tile([128, 2 * D], f32, tag="ps")
                        cols = f3[:, 256 * pi + 1 : 256 * pi + 257].rearrange(
                            "p (m e) -> p e m", e=2
                        )
                        for e in range(2):
                            nc.tensor.matmul(
                                ps[:, D * e : D * (e + 1)],
                                cols[:, e, :],
                                wt[:],
                                start=True,
                                stop=True,
                            )
                        cdst = stage[:, 2 * D * pi : 2 * D * (pi + 1)]
                        if pi % 2 == 0:
                            nc.vector.tensor_copy(cdst, ps[:])
                        else:
                            nc.scalar.copy(cdst, ps[:])
                    deng = (nc.sync, nc.scalar)[b % 2]
                    deng.dma_start(
                        y_d.ap()[
                            b, SOFF[P] : SOFF[P] + nt * 128, :
                        ].rearrange("(pi p e) d -> p pi (e d)", p=128, e=2),
                        stage[:].rearrange("p (pi m) -> p pi m", m=2 * D),
                    )

            # ---------------- pipeline
            wrap_low()
            load_slab(0)
            stats_slab(0)
            relay_slab(0)
            load_slab(1)
            stats_slab(1)
            relay_slab(1)
            f3s0 = [build_f3(b, 0) for b in range(BSH)]
            load_slab(2)
            stats_slab(2)
            relay_slab(2)
            load_slab(3)
            stats_slab(3)
            relay_slab(3)
            f3s1 = [build_f3(b, 1) for b in range(BSH)]
            mm_pair(0, f3s0)
            mm_pair(1, f3s1)

    nc.compile()
    return nc


def _prep_host(W_conv, b_conv):
    import ml_dtypes

    wt = np.empty((K, D), np.float32)
    wkf = np.ascontiguousarray(W_conv.transpose(2, 1, 0)).copy()  # (3, 35, 512)
    wkf[:, C : 2 * C, :] *= 1.0 / W  # fold mean = S24/24 into weights
    # row order within a tap: 5c + t (channel-major, matches F3 gather)
    wkf = wkf.reshape(3, 5, C, D).transpose(0, 2, 1, 3).reshape(3, NF, D)
    wt[: K - 1] = wkf.reshape(3 * NF, D)
    wt[K - 1] = b_conv.astype(np.float32)
    return wt.astype(ml_dtypes.bfloat16)


def _run(x, W_conv, b_conv, trace=False, **kw):
    from concourse import bass_utils

    if "nc" not in _CACHE:
        _CACHE["nc"] = _build()
    nc = _CACHE["nc"]

    wt = _prep_host(np.asarray(W_conv), np.asarray(b_conv))
    import ml_dtypes

    ones = np.ones((1, F3W), ml_dtypes.bfloat16)
    x = np.ascontiguousarray(
        np.asarray(x, np.float32).transpose(0, 2, 1)
    )  # (B, C, S)
    in_maps = [
        {"x": x[BSH * i : BSH * (i + 1)], "wt": wt, "ones": ones}
        for i in range(NCORES)
    ]
    res = bass_utils.run_bass_kernel_spmd(
        nc, in_maps, core_ids=list(range(NCORES)), trace=trace, **kw
    )
    out = np.concatenate(
        [np.asarray(r["y"], np.float32) for r in res.results], axis=0
    )
    return out, res


def kernel(x, x_mark=None, W_conv=None, b_conv=None, **_unused):
    out, _ = _run(x, W_conv, b_conv, trace=False)
    return out
